# revision 11
# baseline (speedup 1.0000x reference)
"""ExpertRouter (MoE routing) Trainium2 Bass kernel.

Distribution: data-parallel over batch*seq. 16384 tokens are split into 8
contiguous shards of 2048 tokens, one per NeuronCore; the tiny gate weight is
replicated. The host hands each core its token shard in transposed layout
[H, 2048] (hidden-major) so the PE needs no on-chip transposes of x.

Per core, per 512-token group: logits^T [64, 512] accumulates over 16
k-chunks of fp32 matmuls with the gate (wT chunk) stationary and xT moving
at N=512 — long moving streams keep the PE's HAM clock gate at 8/8 (pure
N=64 streams measured at ~26% array duty re-throttle the PE to half clock).
The [64, 512] result is copied to SBUF and flipped to [128 tok, 64 e] tiles
with four PE transposes. Then exp + row-sum on ACT (accum_out), top-2 via
DVE max8/max_index8, renormalized weights, and a per-core
sum-of-softmax-probs [64] accumulated on PSUM via a ones-trick matmul. The
host gathers shards, bincounts the indices, finishes the scalar aux loss.
"""

import sys

sys.path.insert(0, "/opt/trn_rl_repo")

import numpy as np

import concourse.bass as bass
import concourse.bacc as bacc
import concourse.mybir as mybir
import concourse.tile as tile
from concourse.bass import ts
from concourse.bass_utils import run_bass_kernel_spmd
from concourse.masks import make_identity

NCORES = 8
B, S, H = 4, 4096, 2048
E = 64
TOKENS = B * S            # 16384
TPC = TOKENS // NCORES    # 2048 tokens per core
NT = TPC // 128           # 16 token tiles per core
KC = H // 128             # 16 contraction chunks
NG = NT // 4              # groups of 512 tokens
WARMUP_MM = 4
TOP_K = 2
COEFF = 0.01

F32 = mybir.dt.float32
U32 = mybir.dt.uint32


def build():
    nc = bacc.Bacc(None, target_bir_lowering=False, debug=True)
    xt_in = nc.declare_dram_parameter("xt", [NG, 128, KC, 512], F32, isOutput=False)
    wt_in = nc.declare_dram_parameter("wt", [H, E], F32, isOutput=False)
    w12_out = nc.declare_dram_parameter("w12", [128, NT * 2], F32, isOutput=True)
    i12_out = nc.declare_dram_parameter("i12", [128, NT * 2], U32, isOutput=True)
    pe_out = nc.declare_dram_parameter("pe", [1, E], F32, isOutput=True)

    with tile.TileContext(nc) as tc:
        with (
            tc.tile_pool(name="const", bufs=1) as cpool,
            tc.tile_pool(name="xts", bufs=3) as xpool,
            tc.tile_pool(name="sa", bufs=2) as sapool,
            tc.tile_pool(name="e", bufs=4) as epool,
            tc.tile_pool(name="small", bufs=1) as spool,
            tc.tile_pool(name="lga_psum", bufs=2, space="PSUM") as lgapool,
            tc.tile_pool(name="lgb_psum", bufs=2, space="PSUM") as lgbpool,
            tc.tile_pool(name="wu_psum", bufs=1, space="PSUM") as wupool,
            tc.tile_pool(name="pr_psum", bufs=1, space="PSUM") as prpool,
        ):
            ident64 = cpool.tile([64, 64], F32)
            make_identity(nc, ident64)

            # PE warmup while the first DMAs land (HAM ramp).
            wu = cpool.tile([128, 512], F32)
            nc.vector.memset(wu, 0.0)
            wu_ps = wupool.tile([1, 512], F32)
            for _ in range(WARMUP_MM):
                nc.tensor.matmul(
                    wu_ps, wu[:, 0:1], wu, start=True, stop=True,
                    skip_group_check=True,
                )

            wt_sb = cpool.tile([128, KC, E], F32)
            nc.sync.dma_start(wt_sb, wt_in[:].rearrange("(c p) e -> p c e", p=128))

            V = spool.tile([128, NT, 8], F32)
            I = spool.tile([128, NT, 8], U32)
            Z = spool.tile([128, NT], F32)
            RZ = spool.tile([128, NT], F32)

            pr = prpool.tile([1, E], F32)
            ACC = spool.tile([128, E], F32)
            ONES = cpool.tile([128, 1], F32)
            nc.vector.memset(ACC, 0.0)
            nc.vector.memset(ONES, 1.0)

            for gg in range(NG):
                # xT for this group: [128 k_local, 16 c, 512 t]; the host
                # lays each group out contiguously per partition row, so this
                # is one 4 MiB DMA with 128 x 32 KB descriptors.
                xts = xpool.tile([128, KC, 512], F32)
                for q in range(4):
                    nc.sync.dma_start(
                        xts[:, 4 * q : 4 * q + 4, :],
                        xt_in[gg][:, 4 * q : 4 * q + 4, :],
                    )

                # logits^T [64 e, 512 t] over all chunks
                lga = lgapool.tile([64, 512], F32)
                for c in range(KC):
                    nc.tensor.matmul(
                        lga,
                        wt_sb[:, c, :],
                        xts[:, c, :],
                        start=(c == 0),
                        stop=(c == KC - 1),
                        skip_group_check=True,
                    )
                sa = sapool.tile([64, 512], F32)
                nc.vector.tensor_copy(sa, lga)

                lgb = lgbpool.tile([128, 4, E], F32)
                for ti in range(4):
                    tt = gg * 4 + ti
                    # [64, 128] -> [128, 64]
                    nc.tensor.matmul(
                        lgb[:, ti, :],
                        sa[:, ts(ti, 128)],
                        ident64,
                        is_transpose=True,
                        start=True,
                        stop=True,
                        skip_group_check=True,
                    )

                    e_t = epool.tile([128, E], F32)
                    nc.scalar.activation(
                        e_t,
                        lgb[:, ti, :],
                        mybir.ActivationFunctionType.Exp,
                        accum_out=Z[:, tt : tt + 1],
                    )

                    nc.vector.max(out=V[:, tt], in_=e_t)
                    nc.vector.max_index(out=I[:, tt], in_max=V[:, tt], in_values=e_t)
                    nc.vector.reciprocal(RZ[:, tt : tt + 1], Z[:, tt : tt + 1])

                    # probs acc[p, e] += e_t[p, e] * rZ[p, tt]  (DVE)
                    pt_t = epool.tile([128, E], F32, tag="pt")
                    nc.vector.tensor_scalar_mul(pt_t, e_t, RZ[:, tt : tt + 1])
                    nc.vector.tensor_add(ACC, ACC, pt_t)

                # Per-group epilogue: w_i = p_i / (p1 + p2 + 1e-9), p_i = v_i/Z
                gs = slice(gg * 4, gg * 4 + 4)
                P12 = spool.tile([128, NT, 2], F32)
                nc.vector.tensor_mul(
                    P12[:, gs],
                    V[:, gs, 0:2],
                    RZ[:, gs, None].to_broadcast([128, 4, 2]),
                )
                SS = spool.tile([128, NT], F32)
                nc.vector.tensor_reduce(
                    SS[:, gs], P12[:, gs], axis=mybir.AxisListType.X,
                    op=mybir.AluOpType.add,
                )
                nc.vector.tensor_scalar_add(SS[:, gs], SS[:, gs], 1e-9)
                RS = spool.tile([128, NT], F32)
                nc.vector.reciprocal(RS[:, gs], SS[:, gs])
                W12 = spool.tile([128, NT, 2], F32)
                nc.vector.tensor_mul(
                    W12[:, gs], P12[:, gs],
                    RS[:, gs, None].to_broadcast([128, 4, 2]),
                )
                nc.sync.dma_start(
                    w12_out[:, gg * 8 : gg * 8 + 8],
                    W12[:, gs].rearrange("p a b -> p (a b)"),
                )
                nc.sync.dma_start(
                    i12_out[:, gg * 8 : gg * 8 + 8].rearrange(
                        "p (a b) -> p a b", b=2
                    ),
                    I[:, gs, 0:2],
                )

            nc.tensor.matmul(pr, ONES, ACC, start=True, stop=True,
                             skip_group_check=True)
            PS = spool.tile([1, E], F32)
            nc.vector.tensor_copy(PS, pr)
            nc.sync.dma_start(pe_out[:], PS)

    if not nc.is_finalized():
        nc.finalize()
    return nc


_NC = None


def _get_nc():
    global _NC
    if _NC is None:
        _NC = build()
    return _NC


def shard_x(x_flat: np.ndarray):
    """[TOKENS, H] -> per-core [NG, 128, KC, 512] group-blocked transposed
    views: element (gg, p, c, t) = x[gg*512 + t, c*128 + p]."""
    x5 = x_flat.reshape(NCORES, NG, 512, KC, 128)
    return [x5[c].transpose(0, 3, 2, 1) for c in range(NCORES)]


def run_cores(x_shards, wt: np.ndarray, **kw):
    nc = _get_nc()
    in_maps = [{"xt": x_shards[c], "wt": wt} for c in range(NCORES)]
    return run_bass_kernel_spmd(nc, in_maps, list(range(NCORES)), **kw)


def kernel(hidden_states: np.ndarray, gate_w: np.ndarray):
    x_flat = np.asarray(hidden_states, dtype=np.float32).reshape(TOKENS, H)
    wt = np.ascontiguousarray(gate_w.T, dtype=np.float32)

    res = run_cores(shard_x(x_flat), wt)

    weights = np.empty((TOKENS, TOP_K), np.float32)
    indices = np.empty((TOKENS, TOP_K), np.int32)
    probs_sum = np.zeros(E, np.float64)
    for c in range(NCORES):
        r = res.results[c]
        # token (within core) = tt*128 + p ; device layout is [p, tt, 2]
        w_c = r["w12"].reshape(128, NT, TOP_K).transpose(1, 0, 2)
        i_c = r["i12"].reshape(128, NT, TOP_K).transpose(1, 0, 2)
        weights[c * TPC : (c + 1) * TPC] = w_c.reshape(TPC, TOP_K)
        indices[c * TPC : (c + 1) * TPC] = i_c.reshape(TPC, TOP_K).astype(np.int32)
        probs_sum += r["pe"][0].astype(np.float64)

    probs_mean = probs_sum / TOKENS
    counts = np.bincount(indices.reshape(-1), minlength=E).astype(np.float64)
    total = float(TOKENS * TOP_K)
    aux = np.float32(COEFF * E * np.sum(counts / (total + 1e-9) * probs_mean))

    return (
        weights.reshape(B, S, TOP_K),
        indices.reshape(B, S, TOP_K),
        aux,
    )
